# revision 18
# baseline (speedup 1.0000x reference)
"""AttnBlock (GroupNorm -> QKV 1x1 -> full NxN attention -> proj -> residual)
for Trainium2, SPMD over 8 NeuronCores.

Sharding: data-parallel over batch (2) x query-pixel blocks (4 of 1024 px).
Each core receives its batch image x [C, N] (for stats/K/V) and its query
slice xq [C, NQ]; K and Vt are computed redundantly per batch pair, queries
are disjoint.  No collectives.

Structure per core:
  1. stream x once for GroupNorm stats (bn_stats/bn_aggr + tiny indicator
     matmuls for the cross-partition group reduce) -> per-channel A, B
  2. stream x again; per chunk materialize hn = A*x + B and matmul into
     resident K [c, n] and V^T [n, c]; Q (with attn scale folded on host
     into wq/bq) for the core's own query block only
  3. attention streamed over 32 k-tiles: S^T[k,q] = K^T@Q -> exp (no max
     subtraction: scores ~ N(0,1), fp32-safe) -> AV accumulated in PSUM
     over all k; softmax denominators accumulated as N=1 matmuls into a
     [128q, QS] psum tile (order-safe via a pre-clearing dummy matmul)
  4. normalize, PE-transpose O^T -> O, proj, + host-folded bias + residual

precision="fp32": exact fp32 matmuls (4 cycles/row on PE).
precision="tf32": float32r matmuls (1 cycle/row; TF32 rounding on operands,
fp32 accumulation).  All matmul operands are produced by compute ops with
float32r output dtype as the BIR verifier requires.
"""

from contextlib import ExitStack

import numpy as np

import concourse.bacc as bacc
import concourse.bass as bass
import concourse.mybir as mybir
import concourse.tile as tile

F32 = mybir.dt.float32
F32R = mybir.dt.float32r
AF = mybir.ActivationFunctionType


def build_program(C=512, G=32, N=4096, NQ=1024, eps=1e-5, precision="fp32"):
    """Emit the per-core Bass program (SPMD; per-core data differs only)."""
    P = 128
    CS = C // P                  # channel subtiles
    KT = N // P                  # key/pixel tiles
    NCH = min(512, N)            # streamed x chunk (pixels); also bn window
    NCHUNKS = N // NCH
    QP = min(512, NQ)            # query-pass width
    QPASSES = NQ // QP
    QS = QP // P                 # query subtiles per pass
    cpg = C // G                 # channels per group
    GPS = P // cpg               # groups per channel-subtile
    assert C % P == 0 and N % P == 0 and NQ % QP == 0 and P % cpg == 0
    MMDT = F32R if precision == "tf32" else F32

    # Bacc layer: its compile() pass legalizes sync waits (<=1 per
    # instruction via EventSemaphore chains), does register allocation, etc.
    nc = bacc.Bacc(None, target_bir_lowering=False)

    x_d = nc.dram_tensor("x", [C, N], F32, kind="ExternalInput")
    xq_d = nc.dram_tensor("xq", [C, NQ], F32, kind="ExternalInput")
    wt_d = {
        w: nc.dram_tensor(f"w{w}t", [C, C], F32, kind="ExternalInput")
        for w in ("q", "k", "v", "p")
    }
    bqT_d = nc.dram_tensor("bqT", [P, CS], F32, kind="ExternalInput")
    bkT_d = nc.dram_tensor("bkT", [P, CS], F32, kind="ExternalInput")
    bpT_d = nc.dram_tensor("bpT", [P, CS], F32, kind="ExternalInput")
    gamma_d = nc.dram_tensor("gamma", [C], F32, kind="ExternalInput")
    beta_d = nc.dram_tensor("beta", [C], F32, kind="ExternalInput")
    indg_d = nc.dram_tensor("indg", [P, GPS], F32, kind="ExternalInput")
    inde_d = nc.dram_tensor("inde", [GPS, P], F32, kind="ExternalInput")
    ident_d = nc.dram_tensor("ident", [P, P], F32, kind="ExternalInput")
    out_d = nc.dram_tensor("out", [C, NQ], F32, kind="ExternalOutput")

    x_r = x_d[:, :].rearrange("(s p) n -> p s n", p=P)
    xq_r = xq_d[:, :].rearrange("(s p) n -> p s n", p=P)
    out_r = out_d[:, :].rearrange("(s p) n -> p s n", p=P)

    with tile.TileContext(nc) as tc, ExitStack() as st:
        const = st.enter_context(tc.tile_pool(name="const", bufs=1))
        big = st.enter_context(tc.tile_pool(name="big", bufs=1))
        small = st.enter_context(tc.tile_pool(name="small", bufs=1))
        # PSUM: shared scratch (3 banks) + O accumulators (QS banks) + sums (1)
        ps_sh = st.enter_context(tc.tile_pool(name="ps_sh", bufs=3, space="PSUM"))
        ps_o = st.enter_context(tc.tile_pool(name="ps_o", bufs=QS, space="PSUM"))
        ps_sum = st.enter_context(tc.tile_pool(name="ps_sum", bufs=1, space="PSUM"))

        # ---- constants / params -------------------------------------------
        indg = const.tile([P, GPS], F32, tag="indg")
        nc.sync.dma_start(out=indg, in_=indg_d[:, :])
        inde = const.tile([GPS, P], F32, tag="inde")
        nc.sync.dma_start(out=inde, in_=inde_d[:, :])
        ident = const.tile([P, P], F32, tag="ident")
        nc.sync.dma_start(out=ident, in_=ident_d[:, :])
        gammaT = const.tile([P, CS], F32, tag="gammaT")
        nc.sync.dma_start(out=gammaT, in_=gamma_d[:].rearrange("(s p) -> p s", p=P))
        betaT = const.tile([P, CS], F32, tag="betaT")
        nc.sync.dma_start(out=betaT, in_=beta_d[:].rearrange("(s p) -> p s", p=P))
        bT = {}
        for name, d in (("q", bqT_d), ("k", bkT_d), ("p", bpT_d)):
            t = const.tile([P, CS], F32, tag=f"bT_{name}")
            nc.sync.dma_start(out=t, in_=d[:, :])
            bT[name] = t
        ones_r = const.tile([P, 1], MMDT, tag="ones_r")
        nc.vector.memset(ones_r, 1.0)
        eps_t = const.tile([P, 1], F32, tag="eps")
        nc.vector.memset(eps_t, eps)

        # resident big tensors (all matmul operands -> MMDT)
        K_sb = big.tile([P, CS, N], MMDT, tag="K")       # K[co, n]
        VT_sb = big.tile([P, KT, C], MMDT, tag="VT")     # V^T[n, co]
        Q_sb = big.tile([P, CS, NQ], MMDT, tag="Q")      # Q[co, nq] (scaled)
        wpT = big.tile([P, CS, C], MMDT, tag="wpT")      # proj weight (phase 3)

        # ---- phase 1: group-norm stats over streamed x --------------------
        with ExitStack() as st1:
            xch = st1.enter_context(tc.tile_pool(name="xch", bufs=2))
            hnp = st1.enter_context(tc.tile_pool(name="hnp", bufs=2))
            wqkv = st1.enter_context(tc.tile_pool(name="wqkv", bufs=1))

            def load_weight(w, pool, tag):
                if pool is None:
                    t = wpT
                else:
                    t = pool.tile([P, CS, C], MMDT, tag=tag, name=f"w_{w}")
                if MMDT is F32:
                    nc.sync.dma_start(
                        out=t,
                        in_=wt_d[w][:, :].rearrange("(s p) c -> p s c", p=P))
                else:
                    raw = xch.tile([P, CS, C], F32, tag="xc", name=f"wraw_{w}")
                    nc.sync.dma_start(
                        out=raw,
                        in_=wt_d[w][:, :].rearrange("(s p) c -> p s c", p=P))
                    nc.vector.tensor_copy(out=t, in_=raw)  # rounds to f32r
                return t

            stats_all = small.tile([P, CS, NCHUNKS, 6], F32, tag="stats")
            for ch in range(NCHUNKS):
                xc = xch.tile([P, CS, NCH], F32, tag="xc")
                nc.sync.dma_start(out=xc, in_=x_r[:, :, ch * NCH:(ch + 1) * NCH])
                for s in range(CS):
                    nc.vector.bn_stats(out=stats_all[:, s, ch, :], in_=xc[:, s, :])
            mv = small.tile([P, CS, 2], F32, tag="mv")
            for s in range(CS):
                nc.vector.bn_aggr(out=mv[:, s, :], in_=stats_all[:, s, :, :])

            # per-channel mean / E[x^2] -> group reduce via indicator matmul
            rhs8 = small.tile([P, 2 * CS], F32, tag="rhs8")
            nc.vector.tensor_copy(out=rhs8[:, 0:CS], in_=mv[:, :, 0])
            nc.vector.tensor_mul(out=rhs8[:, CS:], in0=mv[:, :, 0], in1=mv[:, :, 0])
            nc.vector.tensor_add(out=rhs8[:, CS:], in0=rhs8[:, CS:], in1=mv[:, :, 1])
            ps_g = ps_sh.tile([GPS, 2 * CS], F32, tag="sbank")
            nc.tensor.matmul(ps_g, lhsT=indg, rhs=rhs8, start=True, stop=True)
            gtmp = small.tile([GPS, 2 * CS], F32, tag="gtmp")
            nc.vector.tensor_scalar_mul(gtmp, ps_g, 1.0 / cpg)
            # gvar = E[x^2] - mean^2 ; grstd = 1/sqrt(gvar + eps)
            gsq = small.tile([GPS, CS], F32, tag="gsq")
            nc.vector.tensor_mul(out=gsq, in0=gtmp[:, 0:CS], in1=gtmp[:, 0:CS])
            e8 = small.tile([GPS, 2 * CS], F32, tag="e8")
            nc.vector.tensor_sub(out=e8[:, 0:CS], in0=gtmp[:, CS:], in1=gsq)
            nc.scalar.activation(out=e8[:, 0:CS], in_=e8[:, 0:CS], func=AF.Sqrt,
                                 bias=eps_t[:GPS], scale=1.0)
            nc.vector.reciprocal(out=e8[:, 0:CS], in_=e8[:, 0:CS])
            nc.vector.tensor_copy(out=e8[:, CS:], in_=gtmp[:, 0:CS])
            # expand groups -> channels
            ps_e = ps_sh.tile([P, 2 * CS], F32, tag="sbank")
            nc.tensor.matmul(ps_e, lhsT=inde, rhs=e8, start=True, stop=True)
            A_sb = small.tile([P, CS], F32, tag="A")
            nc.vector.tensor_mul(out=A_sb, in0=ps_e[:, 0:CS], in1=gammaT)
            B_sb = small.tile([P, CS], F32, tag="B")
            nc.vector.tensor_mul(out=B_sb, in0=ps_e[:, CS:], in1=A_sb)
            nc.vector.tensor_sub(out=B_sb, in0=betaT, in1=B_sb)

            # ---- phase 2: hn chunks -> K, V^T, Q (one weight at a time) ----
            def hn_chunk(src_r, ch, width):
                xc = xch.tile([P, CS, width], F32, tag="xc")
                nc.sync.dma_start(out=xc, in_=src_r[:, :, ch * width:(ch + 1) * width])
                hn = hnp.tile([P, CS, width], MMDT, tag="hn")
                for s in range(CS):
                    nc.vector.tensor_scalar(
                        hn[:, s, :], xc[:, s, :],
                        scalar1=A_sb[:, s:s + 1], scalar2=B_sb[:, s:s + 1],
                        op0=mybir.AluOpType.mult, op1=mybir.AluOpType.add,
                    )
                return hn

            wk = load_weight("k", wqkv, "wt")
            for ch in range(NCHUNKS):             # K rows [co-sub, chunk]
                hn = hn_chunk(x_r, ch, NCH)
                for cs in range(CS):
                    ps_k = ps_sh.tile([P, NCH], F32, tag="sbank")
                    for s in range(CS):
                        nc.tensor.matmul(
                            ps_k, lhsT=wk[:, s, cs * P:(cs + 1) * P],
                            rhs=hn[:, s, :],
                            start=(s == 0), stop=(s == CS - 1),
                        )
                    nc.scalar.activation(
                        out=K_sb[:, cs, ch * NCH:(ch + 1) * NCH], in_=ps_k,
                        func=AF.Identity, bias=bT["k"][:, cs:cs + 1], scale=1.0,
                    )
            wv = load_weight("v", wqkv, "wt")
            for ch in range(NCHUNKS):             # V^T rows [pixel-sub, all co]
                hn = hn_chunk(x_r, ch, NCH)
                for ns in range(NCH // P):
                    ps_v = ps_sh.tile([P, C], F32, tag="sbank")
                    for s in range(CS):
                        nc.tensor.matmul(
                            ps_v, lhsT=hn[:, s, ns * P:(ns + 1) * P],
                            rhs=wv[:, s, :],
                            start=(s == 0), stop=(s == CS - 1),
                        )
                    nc.vector.tensor_copy(
                        out=VT_sb[:, ch * (NCH // P) + ns, :], in_=ps_v
                    )
            wq = load_weight("q", wqkv, "wt")
            qw_ = min(NCH, NQ)
            for ch in range(NQ // qw_):           # Q rows (own block only)
                hn = hn_chunk(xq_r, ch, qw_)
                for cs in range(CS):
                    ps_q = ps_sh.tile([P, qw_], F32, tag="sbank")
                    for s in range(CS):
                        nc.tensor.matmul(
                            ps_q, lhsT=wq[:, s, cs * P:(cs + 1) * P],
                            rhs=hn[:, s, :],
                            start=(s == 0), stop=(s == CS - 1),
                        )
                    nc.scalar.activation(
                        out=Q_sb[:, cs, ch * qw_:(ch + 1) * qw_], in_=ps_q,
                        func=AF.Identity, bias=bT["q"][:, cs:cs + 1], scale=1.0,
                    )
            load_weight("p", None, None)

        # ---- phase 3: attention + proj + residual, per query pass ---------
        with ExitStack() as st2:
            ptp = st2.enter_context(tc.tile_pool(name="ptp", bufs=3))
            onp = st2.enter_context(tc.tile_pool(name="onp", bufs=2))
            ocq = st2.enter_context(tc.tile_pool(name="ocq", bufs=1))
            outp = st2.enter_context(tc.tile_pool(name="outp", bufs=2))
            xres = st2.enter_context(tc.tile_pool(name="xres", bufs=2))
            sm2 = st2.enter_context(tc.tile_pool(name="sm2", bufs=2))

            for qp in range(QPASSES):
                q0 = qp * QP
                o_ps = []
                for _qs in range(QS):
                    o_tile = ps_o.tile([P, C], F32, tag="o", name=f"o_{qp}_{_qs}")
                    o_ps.append(o_tile)
                # [128q, QS] denominator accumulator.  The qs==0 kt==0 matmul
                # carries start=True (clears the bank's has_written bits);
                # explicit deps order the other columns' first matmuls after
                # it, so their start=False writes overwrite-then-accumulate.
                sums_ps = ps_sum.tile([P, QS], F32, tag="sums")
                sums_first = None
                for kt in range(KT):
                    s_ps = ps_sh.tile([P, QP], F32, tag="sbank")
                    for s in range(CS):
                        nc.tensor.matmul(
                            s_ps, lhsT=K_sb[:, s, kt * P:(kt + 1) * P],
                            rhs=Q_sb[:, s, q0:q0 + QP],
                            start=(s == 0), stop=(s == CS - 1),
                        )
                    pt = ptp.tile([P, QP], MMDT, tag="pt")
                    nc.scalar.activation(out=pt, in_=s_ps, func=AF.Exp)
                    last = kt == KT - 1
                    for qs in range(QS):
                        mm = nc.tensor.matmul(
                            sums_ps[:, qs:qs + 1],
                            lhsT=pt[:, qs * P:(qs + 1) * P], rhs=ones_r,
                            start=(kt == 0 and qs == 0), stop=last,
                            skip_group_check=True,
                        )
                        if kt == 0:
                            if qs == 0:
                                sums_first = mm
                            else:
                                tile.add_dep_helper(
                                    mm.ins, sums_first.ins, sync=False,
                                    reason="sums bank-clear order",
                                )
                        nc.tensor.matmul(
                            o_ps[qs], lhsT=pt[:, qs * P:(qs + 1) * P],
                            rhs=VT_sb[:, kt, :],
                            start=(kt == 0), stop=last,
                        )
                rec4 = sm2.tile([P, QS], F32, tag="rec4")
                nc.vector.reciprocal(out=rec4, in_=sums_ps)

                oc = ocq.tile([P, CS, QP], MMDT, tag="ocq")
                for qs in range(QS):
                    on = onp.tile([P, C], F32, tag="on")
                    nc.vector.tensor_scalar_mul(on, o_ps[qs], rec4[:, qs:qs + 1])
                    for cs in range(CS):
                        t_ps = ps_sh.tile([P, P], F32, tag="sbank")
                        nc.tensor.transpose(t_ps, on[:, cs * P:(cs + 1) * P], ident)
                        nc.vector.tensor_copy(
                            out=oc[:, cs, qs * P:(qs + 1) * P], in_=t_ps
                        )
                for cs in range(CS):          # proj rows [co-sub, qpass]
                    ps_p = ps_sh.tile([P, QP], F32, tag="sbank")
                    for s in range(CS):
                        nc.tensor.matmul(
                            ps_p, lhsT=wpT[:, s, cs * P:(cs + 1) * P],
                            rhs=oc[:, s, :],
                            start=(s == 0), stop=(s == CS - 1),
                        )
                    xr_t = xres.tile([P, QP], F32, tag="xr")
                    nc.sync.dma_start(out=xr_t, in_=xq_r[:, cs, q0:q0 + QP])
                    ot = outp.tile([P, QP], F32, tag="ot")
                    nc.vector.tensor_scalar_add(ot, ps_p, bT["p"][:, cs:cs + 1])
                    nc.vector.tensor_add(out=ot, in0=ot, in1=xr_t)
                    nc.sync.dma_start(out=out_r[:, cs, q0:q0 + QP], in_=ot)

    nc.finalize()
    return nc


def make_consts(P=128, cpg=16):
    GPS = P // cpg
    indg = np.zeros((P, GPS), np.float32)
    for p in range(P):
        indg[p, p // cpg] = 1.0
    inde = indg.T.copy()
    return {
        "indg": indg,
        "inde": inde,
        "ident": np.eye(P, dtype=np.float32),
    }


_PROGRAM_CACHE = {}


def _get_program(C, G, N, NQ, precision="fp32"):
    key = (C, G, N, NQ, precision)
    if key not in _PROGRAM_CACHE:
        _PROGRAM_CACHE[key] = build_program(C=C, G=G, N=N, NQ=NQ,
                                            precision=precision)
    return _PROGRAM_CACHE[key]


def make_in_maps(x, gn_w, gn_b, q_w, q_b, k_w, k_b, v_w, v_b, proj_w, proj_b,
                 n_cores=8, G=32):
    """Shard full inputs into per-core input maps (biases folded on host)."""
    f = lambda a: np.ascontiguousarray(np.asarray(a, dtype=np.float32))
    x = f(x)
    b, c, h, w = x.shape
    n = h * w
    qblocks = n_cores // b
    nq = n // qblocks
    cs = c // 128
    scale = np.float32(c ** -0.5)
    xf = x.reshape(b, c, n)

    def to_pcs(v):                       # [C] -> [128, CS] (c = 128*s + p)
        return np.ascontiguousarray(np.asarray(v, np.float32).reshape(cs, 128).T)

    common = {
        "wqt": np.ascontiguousarray(f(q_w).T * scale),
        "wkt": f(k_w).T.copy(), "wvt": f(v_w).T.copy(),
        "wpt": f(proj_w).T.copy(),
        "bqT": to_pcs(f(q_b) * scale),
        "bkT": to_pcs(k_b),
        "bpT": to_pcs(f(proj_w) @ f(v_b) + f(proj_b)),
        "gamma": f(gn_w), "beta": f(gn_b),
        **make_consts(cpg=c // G),
    }
    in_maps = []
    for i in range(n_cores):
        bi, qi = divmod(i, qblocks)
        in_maps.append({
            **common,
            "x": xf[bi].copy(),
            "xq": xf[bi][:, qi * nq:(qi + 1) * nq].copy(),
        })
    return in_maps, (b, c, h, w, n, nq, qblocks)


def kernel(x, gn_w, gn_b, q_w, q_b, k_w, k_b, v_w, v_b, proj_w, proj_b):
    from concourse.bass_utils import run_bass_kernel_spmd

    in_maps, (b, c, h, w, n, nq, qblocks) = make_in_maps(
        x, gn_w, gn_b, q_w, q_b, k_w, k_b, v_w, v_b, proj_w, proj_b
    )
    n_cores = 8
    nc = _get_program(C=c, G=32, N=n, NQ=nq)
    res = run_bass_kernel_spmd(nc, in_maps, list(range(n_cores))).results
    out = np.empty((b, c, n), np.float32)
    for i in range(n_cores):
        bi, qi = divmod(i, qblocks)
        out[bi, :, qi * nq:(qi + 1) * nq] = res[i]["out"]
    return out.reshape(b, c, h, w)
